# revision 33
# baseline (speedup 1.0000x reference)
"""Trainium2 Bass kernel for nn_Attention_66932770341587 (MEGA-style block).

Contract: kernel(**inputs) takes FULL unsharded inputs (as in setup_inputs),
returns the FULL [8, 2048, 768] float32 output. Pure data-parallel over batch
across 8 NeuronCores; each core computes one batch element in feature-major
layout. ~630us HW exec (traced), rel err ~3.8e-3 (gate 2e-2); baseline was
799us untraced / 955us traced.

Design:
  - Sort keys are bf16 VALUES (the silu output itself, sign-folded for
    descending columns); no affine quantization or dequantization. The sign
    is restored by folding it into hproj weight rows on the host, so sorted
    bf16 keys feed the hproj matmul directly.
  - All dense matmuls in bf16 (1 col/cycle on PE vs 2 for fp32r).
  - EMA: host-side pole reduction 16 -> R=4 exponentials per channel (greedy
    OMP with closed-form geometric Gram over lags >= C); exact within-block
    FIR (lags 0..k) + exact prev-block FIR correction (lags 1..C-1, taps
    k[l]-k_hat[l]) + reduced-pole state path (lags >= C); C=4 polyphase;
    all diagonal matrices prebuilt on the host and DMA'd (frees the Scalar
    engine from ~700 on-chip diag builds); fp32 scan state via
    tensor_tensor_scan, bf16 state output.
  - Bitonic sort (66 stages, digit-reversed physical layout): every stage's
    min/max tensor_tensor ops are fused across all 6 column groups and
    bit-split (exhaustive search) until every access pattern has <= 2 own
    free dims with unit-stride inner runs, which keeps the DVE in its 2x
    bf16 mode. The first 10 stages run per-group immediately after each
    group's vproj so the sort starts ~8us into the kernel.
  - Emission order interleaves EMA groups and the mxproj (u/r/hx) units into
    the sort stage stream so PE work executes under the DVE-bound sort and
    the PE stays HAM-warm as long as possible.
"""

import numpy as np
from contextlib import ExitStack

import ml_dtypes
import concourse.bass as bass
import concourse.mybir as mybir
import concourse.tile as tile
from concourse import bacc, bass_utils

F32 = mybir.dt.float32
BF16 = mybir.dt.bfloat16
AF = mybir.ActivationFunctionType
OP = mybir.AluOpType

D, L, H = 768, 2048, 768
G = 6                 # 128-partition d-groups
C = 4                 # polyphase block size for EMA state path
NB = L // C           # 512 blocks
LB = 512              # l-block for P1/P3a matmuls
NLB = L // LB
LB3 = 256             # logical l-block of the digit-reversed layout
R_POLES = 4           # reduced EMA pole count

_CACHE = {}
BF = ml_dtypes.bfloat16


# --------------------------- bitonic sort machinery ---------------------------
def _bitonic_stages(n):
    stages = []
    p = 1
    while (1 << p) <= n:
        stages.append(("flip", p))
        c = p - 2
        while c >= 0:
            stages.append(("std", c))
            c -= 1
        p += 1
    return stages


# Digit-reversed storage: logical bit b -> phys weight.
_BITPW = {0: 512, 1: 1024, 2: 64, 3: 128, 4: 256, 5: 8, 6: 16, 7: 32,
          8: 4, 9: 2, 10: 1}
_NBITS = 11


def _merge_dims(entries):
    dims = []
    for step, cnt in entries:
        if dims and dims[-1][0] == step * 2 and (dims[-1][0] > 0) == (step > 0):
            dims[-1] = [step, dims[-1][1] * 2]
            continue
        dims.append([step, cnt])
    return dims


def _build_op(kind, param, fixed):
    if kind == "std":
        c, negset = param, set()
    else:
        c = param - 1
        negset = set(range(c))
    order = sorted((b for b in range(_NBITS) if b != c and b not in fixed),
                   key=lambda b: -_BITPW[b])
    offA = sum(_BITPW[b] * v for b, v in fixed.items())
    offB = _BITPW[c] + offA
    entsA, entsB = [], []
    for b in order:
        pw = _BITPW[b]
        entsA.append((pw, 2))
        if b in negset:
            entsB.append((-pw, 2))
            offB += pw
        else:
            entsB.append((pw, 2))
    return offA, _merge_dims(entsA), offB, _merge_dims(entsB)


def _stage_ops(kind, param):
    """Ops for one stage: list of (offA, dA, offB, dB, fusable). fusable =
    both APs <= 2 free dims, so a [L, G] group dim can be prepended. Finds
    the smallest bit-split whose sub-ops are ALL <= 2 free dims (fusable);
    falls back to <=3-dim per-group ops."""
    import itertools
    c = param if kind == "std" else param - 1
    o = _build_op(kind, param, {})
    if len(o[1]) <= 2 and len(o[3]) <= 2:
        return [(*o, True)]
    bits = [b for b in range(_NBITS) if b != c]
    for r in (1, 2, 3):
        for combo in itertools.combinations(bits, r):
            subs = []
            for vals in itertools.product((0, 1), repeat=r):
                s = _build_op(kind, param, dict(zip(combo, vals)))
                if len(s[1]) > 2 or len(s[3]) > 2:
                    subs = None
                    break
                subs.append((*s, True))
            if subs is not None:
                return subs
    # fallback: single split by c+1, per-group (<=3 dims)
    out = []
    for v in (0, 1):
        s = _build_op(kind, param, {c + 1: v})
        assert len(s[1]) <= 3 and len(s[3]) <= 3, (kind, param)
        out.append((*s, False))
    return out


_STAGE_OPS = [(kind, prm, _stage_ops(kind, prm))
              for kind, prm in _bitonic_stages(L)]


def _emit_sort_stage(nc, cur, oth, ops):
    for offA, dA, offB, dB, fusable in ops:
        if fusable:
            gdim = [[L, G]]
            A_in = bass.AP(tensor=cur.tensor, offset=cur.offset + offA,
                           ap=[cur.ap[0]] + gdim + dA)
            B_in = bass.AP(tensor=cur.tensor, offset=cur.offset + offB,
                           ap=[cur.ap[0]] + gdim + dB)
            A_out = bass.AP(tensor=oth.tensor, offset=oth.offset + offA,
                            ap=[oth.ap[0]] + gdim + dA)
            B_out = bass.AP(tensor=oth.tensor, offset=oth.offset + offB,
                            ap=[oth.ap[0]] + gdim + dB)
            nc.vector.tensor_tensor(out=A_out, in0=A_in, in1=B_in, op=OP.min)
            nc.vector.tensor_tensor(out=B_out, in0=A_in, in1=B_in, op=OP.max)
        else:
            for g in range(G):
                go = g * L
                A_in = bass.AP(tensor=cur.tensor, offset=cur.offset + go + offA,
                               ap=[cur.ap[0]] + dA)
                B_in = bass.AP(tensor=cur.tensor, offset=cur.offset + go + offB,
                               ap=[cur.ap[0]] + dB)
                A_out = bass.AP(tensor=oth.tensor, offset=oth.offset + go + offA,
                                ap=[oth.ap[0]] + dA)
                B_out = bass.AP(tensor=oth.tensor, offset=oth.offset + go + offB,
                                ap=[oth.ap[0]] + dB)
                nc.vector.tensor_tensor(out=A_out, in0=A_in, in1=B_in, op=OP.min)
                nc.vector.tensor_tensor(out=B_out, in0=A_in, in1=B_in, op=OP.max)


# ------------------------------- kernel build -------------------------------
def _build_nc(R=R_POLES):
    NT = 7 * R + 7  # diags/group: 3R z (q^1..3) + 4R corr + 4 FIR + 3 prevFIR
    nc = bacc.Bacc("TRN2", target_bir_lowering=False, debug=False)

    xT = nc.dram_tensor("xT", [D, L], F32, kind="ExternalInput")
    xbfd = nc.dram_tensor("xbfd", [D, L], BF16, kind="ExternalInput")
    wv = nc.dram_tensor("wv", [D, H], BF16, kind="ExternalInput")
    wm = nc.dram_tensor("wm", [D, 3 * D], BF16, kind="ExternalInput")
    wh = nc.dram_tensor("wh", [H, D], BF16, kind="ExternalInput")
    vb = nc.dram_tensor("vb", [D], F32, kind="ExternalInput")
    ub = nc.dram_tensor("ub", [D], F32, kind="ExternalInput")
    rb = nc.dram_tensor("rb", [D], F32, kind="ExternalInput")
    hxb = nc.dram_tensor("hxb", [D], F32, kind="ExternalInput")
    sgnd = nc.dram_tensor("sgnd", [D], F32, kind="ExternalInput")
    identd = nc.dram_tensor("identd", [128, 128], BF16, kind="ExternalInput")
    diagsd = nc.dram_tensor("diagsd", [G, NT, 128, 128], BF16, kind="ExternalInput")
    mtd = nc.dram_tensor("mtd", [G, 128, R * NB], F32, kind="ExternalInput")
    y = nc.dram_tensor("y", [D, L], F32, kind="ExternalOutput")

    def gp(t):  # [D] DRAM -> [128 part, G] view
        return t.ap().rearrange("(g p) -> p g", p=128)

    with tile.TileContext(nc) as tc, ExitStack() as root:
        dram = root.enter_context(tc.tile_pool(name="dram", bufs=1, space="DRAM"))
        u_d = dram.tile([D, L], BF16)
        hx_d = dram.tile([D, L], BF16)

        persist = root.enter_context(tc.tile_pool(name="persist", bufs=1))
        keys = persist.tile([128, G, L], BF16)
        scratch = persist.tile([128, G, L], BF16)
        r_sb = persist.tile([128, G, L], BF16)
        prm = persist.tile([128, 8, G], F32)
        ident = persist.tile([128, 128], BF16)
        mid = root.enter_context(ExitStack())
        mxpool = mid.enter_context(tc.tile_pool(name="mxp", bufs=1))
        mx = mxpool.tile([128, G, L], BF16)


        with ExitStack() as p12:
            xpool = p12.enter_context(tc.tile_pool(name="xbf", bufs=1))
            x_bf = xpool.tile([128, G, L], BF16)
            wv_stack = ExitStack()
            wvp = wv_stack.enter_context(tc.tile_pool(name="wv", bufs=1))
            wv_sb = wvp.tile([128, G, H], BF16)
            nc.sync.dma_start(out=wv_sb, in_=wv.ap().rearrange("(g p) h -> p g h", p=128))
            for g in range(G):
                nc.sync.dma_start(out=x_bf[:, g, :],
                                  in_=xbfd.ap()[g * 128:(g + 1) * 128, :])
            nc.sync.dma_start(out=ident, in_=identd.ap())
            nc.sync.dma_start(out=prm[:, 0, :], in_=gp(vb))
            nc.sync.dma_start(out=prm[:, 1, :], in_=gp(ub))
            nc.sync.dma_start(out=prm[:, 2, :], in_=gp(rb))
            nc.sync.dma_start(out=prm[:, 3, :], in_=gp(hxb))
            nc.sync.dma_start(out=prm[:, 4, :], in_=gp(sgnd))

            # ---------------- P1: vproj + keys for ALL groups ----------------
            with ExitStack() as p1:
                vpool = p1.enter_context(tc.tile_pool(name="v", bufs=2))
                vps = p1.enter_context(tc.tile_pool(name="vps", bufs=2, space="PSUM"))
                # HAM warm-up: keep PE busy during the initial DMA wait so
                # vproj runs at 2.4GHz instead of 1.2
                wup = p1.enter_context(tc.tile_pool(name="wup", bufs=1))
                w_in = wup.tile([128, LB], BF16)
                nc.gpsimd.memset(w_in[:, :], 0.0)
                wups = p1.enter_context(tc.tile_pool(name="wups", bufs=1, space="PSUM"))
                w_ps = wups.tile([128, LB], F32)
                for _ in range(50):
                    nc.tensor.matmul(out=w_ps, lhsT=w_in[:, 0:128], rhs=w_in,
                                     start=True, stop=True)
                EARLY = 10
                for g in range(G):
                    v_g = vpool.tile([128, L], BF16, tag="v")
                    for lb in range(NLB):
                        ps = vps.tile([128, LB], F32)
                        for k in range(G):
                            nc.tensor.matmul(
                                out=ps,
                                lhsT=wv_sb[:, k, g * 128:(g + 1) * 128],
                                rhs=x_bf[:, k, lb * LB:(lb + 1) * LB],
                                start=(k == 0), stop=(k == G - 1))
                        nc.scalar.activation(out=v_g[:, lb * LB:(lb + 1) * LB],
                                             in_=ps, func=AF.Silu,
                                             bias=prm[:, 0, g:g + 1], scale=1.0)
                    nc.scalar.activation(out=keys[:, g, :], in_=v_g,
                                         func=AF.Identity, scale=prm[:, 4, g:g + 1])
                    curg, othg = keys[:, g, :], scratch[:, g, :]
                    for kind_e, prm_e, ops_e in _STAGE_OPS[:EARLY]:
                        for offA, dA, offB, dB, _f in ops_e:
                            A_in = bass.AP(tensor=curg.tensor,
                                           offset=curg.offset + offA,
                                           ap=[curg.ap[0]] + dA)
                            B_in = bass.AP(tensor=curg.tensor,
                                           offset=curg.offset + offB,
                                           ap=[curg.ap[0]] + dB)
                            A_out = bass.AP(tensor=othg.tensor,
                                            offset=othg.offset + offA,
                                            ap=[othg.ap[0]] + dA)
                            B_out = bass.AP(tensor=othg.tensor,
                                            offset=othg.offset + offB,
                                            ap=[othg.ap[0]] + dB)
                            nc.vector.tensor_tensor(out=A_out, in0=A_in, in1=B_in,
                                                    op=OP.min)
                            nc.vector.tensor_tensor(out=B_out, in0=A_in, in1=B_in,
                                                    op=OP.max)
                        curg, othg = othg, curg
                    assert curg.tensor is keys.tensor

            # -------- P2 (EMA) interleaved into the sort emission stream --------
            wv_stack.close()
            wmp = p12.enter_context(tc.tile_pool(name="wm", bufs=1))
            wm_sb = wmp.tile([128, G, 3 * D], BF16)
            nc.sync.dma_start(out=wm_sb, in_=wm.ap().rearrange("(g p) o -> p g o", p=128))
            ema_stack = ExitStack()
            dpool = ema_stack.enter_context(tc.tile_pool(name="diag", bufs=2))
            mtpool = ema_stack.enter_context(tc.tile_pool(name="mt", bufs=2))
            spool = ema_stack.enter_context(tc.tile_pool(name="scan", bufs=2))
            zps = ema_stack.enter_context(tc.tile_pool(name="zps", bufs=1, space="PSUM"))
            cps = ema_stack.enter_context(tc.tile_pool(name="cps", bufs=1, space="PSUM"))

            def emit_ema(g):
                dg = dpool.tile([128, NT, 128], BF16, tag="dg")
                nc.sync.dma_start(out=dg,
                                  in_=diagsd.ap()[g].rearrange("t p c -> p t c"))
                mt = mtpool.tile([128, R * NB], F32, tag="mt")
                nc.sync.dma_start(out=mt, in_=mtd.ap()[g])

                def xs(off, n=NB):
                    base = x_bf[:, g, :]
                    return bass.AP(tensor=base.tensor, offset=base.offset + off,
                                   ap=[base.ap[0], [C, n]])

                zt = zps.tile([128, R * NB], F32, tag="z")
                for r in range(R):
                    for j in range(C):
                        lhsT = ident if j == 0 else dg[:, 3 * r + (j - 1), :]
                        nc.tensor.matmul(out=zt[:, r * NB:(r + 1) * NB],
                                         lhsT=lhsT, rhs=xs(C - 1 - j),
                                         start=(j == 0), stop=(j == C - 1))
                stile = spool.tile([128, R, NB + 1], BF16, tag="s")
                nc.vector.memset(
                    bass.AP(tensor=stile.tensor, offset=stile.offset,
                            ap=[stile.ap[0], [NB + 1, R], [1, 1]]), 0.0)
                for r in range(R):
                    nc.vector.tensor_tensor_scan(
                        out=stile[:, r, 1:NB + 1],
                        data0=mt[:, r * NB:(r + 1) * NB],
                        data1=zt[:, r * NB:(r + 1) * NB],
                        initial=0.0, op0=OP.mult, op1=OP.add)
                conv = cps.tile([128, C, NB], F32, tag="conv")
                for k in range(C):
                    for j in range(k + 1):          # within-block FIR, exact
                        nc.tensor.matmul(out=conv[:, k, :],
                                         lhsT=dg[:, 7 * R + j, :], rhs=xs(k - j),
                                         start=(j == 0), stop=False)
                    for m in range(k + 1, C):       # prev-block FIR corr, lags<C
                        lag = C + k - m
                        pf = bass.AP(tensor=x_bf.tensor,
                                     offset=x_bf[:, g, :].offset + m,
                                     ap=[x_bf.ap[0], [C, NB - 1]])
                        nc.tensor.matmul(out=conv[:, k, 1:NB],
                                         lhsT=dg[:, 7 * R + 4 + (lag - 1), :],
                                         rhs=pf, start=False, stop=False)
                    for r in range(R):              # reduced-pole states
                        nc.tensor.matmul(out=conv[:, k, :],
                                         lhsT=dg[:, 3 * R + 4 * r + k, :],
                                         rhs=stile[:, r, 0:NB],
                                         start=False, stop=(r == R - 1))
                for k in range(C):
                    mo = bass.AP(tensor=mx.tensor, offset=mx.offset + g * L + k,
                                 ap=[mx.ap[0], [C, NB]])
                    nc.scalar.activation(out=mo, in_=conv[:, k, :], func=AF.Silu)

            # ---- unified emission: sort stages with EMA (stages 0..5) and
            # P3a mxproj (after EMA pools close) interleaved for PE warmth ----
            EMA_DONE = 21
            p3a_stack = ExitStack()
            p3a_state = {}

            def open_p3a():
                ev = p3a_stack.enter_context(tc.tile_pool(name="ev", bufs=4))
                mps = p3a_stack.enter_context(
                    tc.tile_pool(name="mps", bufs=4, space="PSUM"))
                p3a_state.update(ev=ev, mps=mps)
                warm = p3a_stack.enter_context(
                    tc.tile_pool(name="warm", bufs=1, space="PSUM"))
                warm_ps = warm.tile([128, LB], F32, tag="warm")
                warm_ref["ps"] = warm_ps

            def emit_p3a_unit(lb, t, g):
                ev, mps = p3a_state["ev"], p3a_state["mps"]
                o = t * G + g
                ps = mps.tile([128, LB], F32)
                for k in range(G):
                    nc.tensor.matmul(
                        out=ps,
                        lhsT=wm_sb[:, k, o * 128:(o + 1) * 128],
                        rhs=mx[:, k, lb * LB:(lb + 1) * LB],
                        start=(k == 0), stop=(k == G - 1))
                if t == 0:
                    e = ev.tile([128, LB], BF16, tag="ev")
                    nc.scalar.activation(out=e, in_=ps, func=AF.Sigmoid,
                                         bias=prm[:, 1, g:g + 1], scale=1.0)
                    nc.sync.dma_start(
                        out=u_d[g * 128:(g + 1) * 128, lb * LB:(lb + 1) * LB],
                        in_=e)
                elif t == 1:
                    nc.scalar.activation(out=r_sb[:, g, lb * LB:(lb + 1) * LB],
                                         in_=ps, func=AF.Silu,
                                         bias=prm[:, 2, g:g + 1], scale=1.0)
                else:
                    e = ev.tile([128, LB], BF16, tag="ev")
                    nc.scalar.activation(out=e, in_=ps, func=AF.Identity,
                                         bias=prm[:, 3, g:g + 1], scale=1.0)
                    nc.sync.dma_start(
                        out=hx_d[g * 128:(g + 1) * 128, lb * LB:(lb + 1) * LB],
                        in_=e)

            units = [(lb, t, g) for lb in range(NLB)
                     for t in range(3) for g in range(G)]
            NS = len(_STAGE_OPS)
            cur, oth = keys[:, :, :], scratch[:, :, :]
            uidx = 0
            warm_ref = {}
            for si, (kind, prm_, ops) in enumerate(_STAGE_OPS):
                if si < EARLY:
                    continue
                if si < EARLY + G:
                    emit_ema(si - EARLY)
                if si == EMA_DONE:
                    ema_stack.close()
                    open_p3a()
                if si > EMA_DONE:
                    tgt = (si - EMA_DONE) * len(units) // (NS - 1 - EMA_DONE)
                    while uidx < tgt:
                        emit_p3a_unit(*units[uidx])
                        uidx += 1
                _emit_sort_stage(nc, cur, oth, ops)
                cur, oth = oth, cur
            while uidx < len(units):
                emit_p3a_unit(*units[uidx])
                uidx += 1
            p3a_stack.close()
            assert cur.tensor is keys.tensor

        mid.close()  # free mx before P3b allocations

        # ---------------- P3b: t1 = sorted*r, hproj(+hx), h, y ----------------
        # paired 256-blocks -> 512-col hproj matmuls
        with ExitStack() as p3b:
            whp = p3b.enter_context(tc.tile_pool(name="wh", bufs=1))
            wh_sb = whp.tile([128, G, D], BF16)
            nc.sync.dma_start(out=wh_sb, in_=wh.ap().rearrange("(g p) o -> p g o", p=128))
            inp = p3b.enter_context(tc.tile_pool(name="p3in", bufs=2))
            t1p = p3b.enter_context(tc.tile_pool(name="t1", bufs=2))
            hp = p3b.enter_context(tc.tile_pool(name="h", bufs=2))
            yp = p3b.enter_context(tc.tile_pool(name="y", bufs=2))
            hps = p3b.enter_context(tc.tile_pool(name="hps", bufs=2, space="PSUM"))
            PLB = 2 * LB3  # 512
            for pb in range(L // PLB):
                sl = slice(pb * PLB, (pb + 1) * PLB)
                u_sl = inp.tile([128, G, PLB], BF16, tag="u")
                hx_sl = inp.tile([128, G, PLB], BF16, tag="hx")
                x_sl = inp.tile([128, G, PLB], F32, tag="x")
                nc.sync.dma_start(
                    out=u_sl, in_=u_d[:, sl].rearrange("(g p) l -> p g l", p=128))
                nc.sync.dma_start(
                    out=hx_sl, in_=hx_d[:, sl].rearrange("(g p) l -> p g l", p=128))
                nc.sync.dma_start(
                    out=x_sl, in_=xT.ap().rearrange("(g p) l -> p g l", p=128)[:, :, sl])
                t1 = t1p.tile([128, G, PLB], BF16, tag="t1")
                for g in range(G):
                    kg = keys[:, g, :]
                    for h in range(2):
                        lb = 2 * pb + h
                        koff = 4 * (lb & 1) + 2 * ((lb >> 1) & 1) + ((lb >> 2) & 1)
                        kperm = bass.AP(tensor=kg.tensor, offset=kg.offset + koff,
                                        ap=[kg.ap[0], [8, 8], [64, 8], [512, 4]])
                        tout = t1[:, g, h * LB3:(h + 1) * LB3].rearrange(
                            "p (a b c) -> p a b c", a=8, b=8, c=4)
                        rg = r_sb[:, g, lb * LB3:(lb + 1) * LB3].rearrange(
                            "p (a b c) -> p a b c", a=8, b=8, c=4)
                        nc.vector.tensor_tensor(out=tout, in0=kperm, in1=rg, op=OP.mult)
                h_t = hp.tile([128, G, PLB], BF16, tag="h")
                for gh in range(2):          # half the groups per PSUM tile
                    ps = hps.tile([128, 3, PLB], F32)
                    for gi in range(3):
                        g = 3 * gh + gi
                        for k in range(G):
                            nc.tensor.matmul(
                                out=ps[:, gi, :],
                                lhsT=wh_sb[:, k, g * 128:(g + 1) * 128],
                                rhs=t1[:, k, :],
                                start=(k == 0), stop=False)
                        nc.tensor.matmul(out=ps[:, gi, :], lhsT=ident,
                                         rhs=hx_sl[:, g, :], start=False, stop=True)
                    nc.scalar.activation(out=h_t[:, 3 * gh:3 * gh + 3, :], in_=ps,
                                         func=AF.Silu)
                y_t = yp.tile([128, G, PLB], F32, tag="y")
                nc.vector.tensor_tensor(out=y_t, in0=h_t, in1=x_sl, op=OP.subtract)
                nc.vector.tensor_tensor(out=y_t, in0=y_t, in1=u_sl, op=OP.mult)
                nc.vector.tensor_tensor(out=y_t, in0=y_t, in1=x_sl, op=OP.add)
                nc.sync.dma_start(
                    out=y.ap().rearrange("(g p) l -> p g l", p=128)[:, :, sl],
                    in_=y_t)

    nc.finalize()
    return nc


# ------------------------------- host prep -------------------------------
def _pole_reduce(q, w, R):
    """Greedy OMP per channel, closed-form Gram over tail j>=C."""
    Dn, Nn = q.shape
    q = q.astype(np.float64)
    w = w.astype(np.float64)

    def cross(qa, qb):
        x = np.clip(qa * qb, 0.0, 1.0 - 1e-18)
        return (x**C - x**L) / (1.0 - x)

    Gm = cross(q[:, :, None], q[:, None, :])
    bvec = np.einsum("dnm,dm->dn", Gm, w)
    kk = np.einsum("dn,dn->d", w, bvec)
    sel = np.zeros((Dn, 0), dtype=np.int64)
    best_err = kk.copy()
    whv = None
    for r in range(R):
        best_gain = np.full(Dn, -np.inf)
        best_idx = np.zeros(Dn, dtype=np.int64)
        for cand in range(Nn):
            idx = np.concatenate([sel, np.full((Dn, 1), cand, np.int64)], axis=1)
            Gs = np.take_along_axis(
                np.take_along_axis(Gm, idx[:, :, None], 1), idx[:, None, :], 2)
            bs = np.take_along_axis(bvec, idx, 1)
            Gs = Gs + np.eye(r + 1)[None] * 1e-9
            wh_c = np.linalg.solve(Gs, bs[..., None])[..., 0]
            res = kk - np.einsum("dr,dr->d", wh_c, bs)
            gain = best_err - res
            upd = gain > best_gain
            best_gain[upd] = gain[upd]
            best_idx[upd] = cand
        sel = np.concatenate([sel, best_idx[:, None]], axis=1)
        Gs = np.take_along_axis(
            np.take_along_axis(Gm, sel[:, :, None], 1), sel[:, None, :], 2)
        bs = np.take_along_axis(bvec, sel, 1)
        Gs = Gs + np.eye(r + 1)[None] * 1e-9
        whv = np.linalg.solve(Gs, bs[..., None])[..., 0]
        best_err = np.maximum(kk - np.einsum("dr,dr->d", whv, bs), 0.0)
    qh = np.take_along_axis(q, sel, 1)
    return qh, whv


def _host_prep(inputs, R=R_POLES):
    ZD = 192
    x = np.asarray(inputs["x"], np.float32)
    delta = np.asarray(inputs["delta"], np.float64)[..., 0]
    alpha = np.asarray(inputs["alpha"], np.float64)[..., 0]
    beta = np.asarray(inputs["beta"], np.float64)[..., 0]
    gamma = np.asarray(inputs["gamma"], np.float64)
    omega = np.asarray(inputs["omega"], np.float64)
    col_desc = np.asarray(inputs["col_descend"])

    p = 1.0 / (1.0 + np.exp(-delta))
    q = 1.0 - p / (1.0 + np.exp(-alpha))
    w = p * beta * gamma / np.sqrt(gamma.shape[1])

    jj = np.arange(C, dtype=np.float64)
    kf = np.einsum("dn,dnj->dj", w, q[:, :, None] ** jj[None, None, :])
    kf[:, 0] += omega

    qh, wh_p = _pole_reduce(q, w, R)
    # prev-block FIR correction: delta[l] = k[l] - k_hat[l], l = 1..C-1
    dl = np.zeros((D, C))
    for lag in range(1, C):
        dl[:, lag] = (np.einsum("dn,dn->d", w, q**lag)
                      - np.einsum("dr,dr->d", wh_p, qh**lag))

    NT = 7 * R + 7
    diags = np.zeros((G, NT, 128, 128), dtype=BF)
    idx = np.arange(128)
    for g in range(G):
        s = slice(g * 128, (g + 1) * 128)
        for r in range(R):
            for j in range(1, C):
                diags[g, 3 * r + (j - 1), idx, idx] = (qh[s, r] ** j).astype(BF)
            for k in range(C):
                diags[g, 3 * R + 4 * r + k, idx, idx] = (
                    wh_p[s, r] * qh[s, r] ** (k + 1)).astype(BF)
        for j in range(C):
            diags[g, 7 * R + j, idx, idx] = kf[s, j].astype(BF)
        for lag in range(1, C):
            diags[g, 7 * R + 4 + (lag - 1), idx, idx] = dl[s, lag].astype(BF)

    mt = np.zeros((G, 128, R * NB), dtype=np.float32)
    for g in range(G):
        s = slice(g * 128, (g + 1) * 128)
        for r in range(R):
            mt[g, :, r * NB:(r + 1) * NB] = (qh[s, r] ** C)[:, None].astype(np.float32)
    sgn = np.where(col_desc, -1.0, 1.0).astype(np.float32)

    mw = np.asarray(inputs["mxproj_w"], np.float32)
    mb = np.asarray(inputs["mxproj_b"], np.float32)
    wm_f = np.concatenate([mw[0:D], mw[D + ZD:D + ZD + H], mw[D + ZD + H:]], 0)
    wh_f = np.asarray(inputs["hproj_w"], np.float32) * sgn[None, :]

    eye = np.eye(128, dtype=BF)
    shared = dict(
        wv=np.ascontiguousarray(np.asarray(inputs["vproj_w"], np.float32).T).astype(BF),
        wm=np.ascontiguousarray(wm_f.T).astype(BF),
        wh=np.ascontiguousarray(wh_f.T).astype(BF),
        vb=np.asarray(inputs["vproj_b"], np.float32),
        ub=mb[0:D].copy(),
        rb=mb[D + ZD:D + ZD + H].copy(),
        hxb=(mb[D + ZD + H:] + np.asarray(inputs["hproj_b"], np.float32)),
        sgnd=sgn, identd=eye, diagsd=diags, mtd=mt,
    )
    xT = np.ascontiguousarray(x.transpose(0, 2, 1))
    return shared, xT


def kernel(**inputs):
    if "nc" not in _CACHE:
        _CACHE["nc"] = _build_nc()
    nc = _CACHE["nc"]
    shared, xT = _host_prep(inputs)
    B = xT.shape[0]
    in_maps = [dict(shared, xT=np.ascontiguousarray(xT[b]),
                    xbfd=np.ascontiguousarray(xT[b]).astype(BF)) for b in range(B)]
    res = bass_utils.run_bass_kernel_spmd(
        nc, in_maps, core_ids=list(range(B)),
        trace=bool(_CACHE.get("trace", False)))
    _CACHE["last_result"] = res
    out = np.stack([res.results[b]["y"].reshape(D, L).T for b in range(B)])
    return np.ascontiguousarray(out.astype(np.float32))
